# revision 10
# baseline (speedup 1.0000x reference)
"""Mixtral attention layer (B=1, S=2048, H=4096, NH=32, NKV=8, HD=128) on 8
Trainium2 NeuronCores, tensor-parallel over heads (core c owns 4 query heads
and 1 KV head: column-shard of wq/wk/wv, row-shard of wo; the host sums the
8 bf16 partials and adds the residual).

Per-core pipeline:
  Projections: 3-term e4m3 decomposition x ~ (x8+dx8)/8, w ~ (w8+dw8)/2048,
    with the deltas stored at the SAME scale as the base terms (e4m3
    relative precision is scale-invariant for normals).  Per 256-row
    contraction chunk, 3 fp8 DoubleRow matmuls (w8*x8, dw8*x8, w8*dx8) at
    0.5 cycles/row accumulate the product (accurate to ~0.1%, better than
    bf16) in PSUM at value scale 16384, for 0.75x the PE cost of bf16.
    Two sweeps: {k, v, q0, q1} then {q2, q3}, so sweep-1 evacuations and
    the v transposes overlap sweep-2 matmuls and PSUM stays at <= 6 banks.
  Norm stats ride inside sweep 1: x2 = 8*x^2 in e4m3 (ACT Square / DVE
    scalar_tensor_tensor, alternating), reduced over H by an all-ones
    DoubleRow matmul into a [128,ST] PSUM whose partitions are all equal;
    R = r/16384 = exp(-0.5*ln(scale*sum+bias)) -- ln+exp live in the same
    ACT table set as Copy/Square so no table reloads (Sqrt would force
    two per tile).  R folds into bf16 RoPE tables (cp=cos*R, sp=sin_sw*R).
  RoPE evac: ACT copy PSUM->bf16, then 3 DVE table ops; q^T (4 heads) and
    k^T stay SBUF-resident bf16; v^T is multiplied by R and PE-transposed
    per 128-chunk into v-natural [j,d] bf16 (the AV stationary).
  Attention per (head, i-tile of 512): causal flash with diagonal
    trimming -- for diagonal j-chunks only the valid i-subrange is
    computed (scores matmul, exp, AV, Z all on [off:512]), and the
    triangle mask is a single 128-wide affine_select.  scores^T =
    k_chunk.T @ q (bf16, 1 cycle/row), exp on ACT -> p bf16, AV
    accumulates with v-natural stationary, Z row-sums via an all-ones
    bf16 stationary whose [128,ST] output is partition-broadcast for
    free; attn^T = av * (1/z) -> bf16.
  o_proj: same 3-term fp8 DoubleRow delta trick as the projections, with
    the attention output produced on device as an fp8 pair: a8 =
    e4m3(32*attn) (the 32x range scale folded into the Z-ones constant),
    d8 = e4m3(attn_bf16 - a8) -- the pair represents the bf16 value to
    ~0.01%, so precision matches the bf16 path at 0.75x the PE cost.
    Contraction runs over (head-pair, d) 256-chunks against host-prepared
    wo8/dwo8; PSUM evac is a scaled copy (1/65536) alternating DVE/ACT ->
    bf16 staging -> DMA out.  o_proj chunks of tile st-1 are emitted
    before each head of tile st's attention so their always-ready matmuls
    fill PE stalls while attention waits on ACT exp.
  DMA: tile-0 x8/dx8 chunk groups ride the ACT queue (parallel with the
    SP weight stream at startup); later tiles use SP, whose sequencer is
    idle during attention -- an ACT-queue DMA dispatch costs 667ns of the
    same sequencer that paces exp.  Weights on SP in need-order with the
    leading wq/dwq quarters split for fast first-chunk arrival; wo last.
"""

import math

import numpy as np

import concourse.bass as bass
import concourse.tile as tile
from concourse import bacc, mybir
from concourse.masks import make_identity

F32 = mybir.dt.float32
BF16 = mybir.dt.bfloat16
E4 = mybir.dt.float8e4

# Full problem dims
B, S, H, NH, NKV, HD = 1, 2048, 4096, 32, 8, 128
EPS = 1e-5
N_CORES = 8
QH = NH // N_CORES           # query heads per core = 4
DQ = QH * HD                 # q columns per core = 512
DKV = (NKV // N_CORES) * HD  # kv columns per core = 128

# x ~ (x8 + dx8)/SX, w ~ (w8 + dw8)/SW.  The deltas are stored at the SAME
# scale as the base terms (e4m3 relative precision is scale-invariant for
# normals; residuals of tiny values land subnormal and contribute nothing),
# so the three DoubleRow products accumulate in PSUM without per-term
# compensation: PSUM = SX*SW*(x*w + x*dw + dx*w).
SX, FX = 8.0, 1.0
SW, FW = 2048.0, 1.0
PS_SCALE = SX * SW           # PSUM projection value scale = 16384
DR = mybir.MatmulPerfMode.DoubleRow


def build_bass(s=S, h=H, qh=QH, stop_after=None, no_trim=False, debug=False):
    ST = 512                   # s-tile width
    NST = s // ST              # 4 s-tiles
    HC2 = h // 256             # 16 double-row contraction chunks
    XG = 4                     # x-chunk DMA group (4 hc chunks per DMA)
    NJ = s // 128              # 16 key chunks
    NHT = h // 512             # 8 o_proj column tiles
    scale = 1.0 / math.sqrt(HD)

    nc = bacc.Bacc(None, target_bir_lowering=False)

    x8d = nc.dram_tensor("x8", [128, HC2, 2, s], E4, kind="ExternalInput")
    dx8d = nc.dram_tensor("dx8", [128, HC2, 2, s], E4, kind="ExternalInput")
    wq8d = nc.dram_tensor("wq8", [128, HC2, 2, DQ], E4, kind="ExternalInput")
    dwq8d = nc.dram_tensor("dwq8", [128, HC2, 2, DQ], E4, kind="ExternalInput")
    wk8d = nc.dram_tensor("wk8", [128, HC2, 2, DKV], E4, kind="ExternalInput")
    dwk8d = nc.dram_tensor("dwk8", [128, HC2, 2, DKV], E4, kind="ExternalInput")
    wv8d = nc.dram_tensor("wv8", [128, HC2, 2, DKV], E4, kind="ExternalInput")
    dwv8d = nc.dram_tensor("dwv8", [128, HC2, 2, DKV], E4, kind="ExternalInput")
    wo8d = nc.dram_tensor("wo8", [128, 2, 2, h], E4, kind="ExternalInput")
    dwo8d = nc.dram_tensor("dwo8", [128, 2, 2, h], E4, kind="ExternalInput")
    cosd = nc.dram_tensor("cosT", [128, s], BF16, kind="ExternalInput")
    sind = nc.dram_tensor("sinTs", [128, s], BF16, kind="ExternalInput")
    outd = nc.dram_tensor("out", [s, h], BF16, kind="ExternalOutput")
    if debug:
        dbg = {
            "dbg_qT": nc.dram_tensor("dbg_qT", [128, qh, s], BF16,
                                     kind="ExternalOutput"),
            "dbg_kT": nc.dram_tensor("dbg_kT", [128, s], BF16,
                                     kind="ExternalOutput"),
            "dbg_vT": nc.dram_tensor("dbg_vT", [128, s], BF16,
                                     kind="ExternalOutput"),
            "dbg_vnat": nc.dram_tensor("dbg_vnat", [128, s // 128, 128], BF16,
                                       kind="ExternalOutput"),
            "dbg_R": nc.dram_tensor("dbg_R", [128, s], F32,
                                    kind="ExternalOutput"),
        }

    with tile.TileContext(nc) as tc:
        with (
            tc.tile_pool(name="persist", bufs=1) as persist,
            tc.tile_pool(name="xin", bufs=4) as xin,
            tc.tile_pool(name="x2p", bufs=3) as x2p,
            tc.tile_pool(name="tabs", bufs=2) as tabs,
            tc.tile_pool(name="evacp", bufs=2) as evacp,
            tc.tile_pool(name="probs", bufs=6) as probs,
            tc.tile_pool(name="zrp", bufs=2) as zrp,
            tc.tile_pool(name="outp", bufs=3) as outp,
            tc.tile_pool(name="acc_ps", bufs=8, space="PSUM") as acc_ps,
        ):
            # ---- persistent SBUF ----
            wq8 = persist.tile([128, HC2, 2, DQ], E4, tag="wq8")
            dwq8 = persist.tile([128, HC2, 2, DQ], E4, tag="dwq8")
            wk8 = persist.tile([128, HC2, 2, DKV], E4, tag="wk8")
            dwk8 = persist.tile([128, HC2, 2, DKV], E4, tag="dwk8")
            wv8 = persist.tile([128, HC2, 2, DKV], E4, tag="wv8")
            dwv8 = persist.tile([128, HC2, 2, DKV], E4, tag="dwv8")
            wo8_sb = persist.tile([128, 2, 2, h], E4, tag="wo8")
            dwo8_sb = persist.tile([128, 2, 2, h], E4, tag="dwo8")
            cos_sb = persist.tile([128, s], BF16, tag="cos")
            sin_sb = persist.tile([128, s], BF16, tag="sin")
            qT = persist.tile([128, qh, s], BF16, tag="qT")
            kT = persist.tile([128, s], BF16, tag="kT")
            vT = persist.tile([128, s], BF16, tag="vT")
            vnat = persist.tile([128, NJ, 128], BF16, tag="vnat")
            attnT8 = persist.tile([128, 2, 2, s], E4, tag="attnT8")
            dattnT8 = persist.tile([128, 2, 2, s], E4, tag="dattnT8")
            ones8 = persist.tile([128, 2, 128], E4, tag="ones8")
            ones_bf = persist.tile([128, 128], BF16, tag="ones_bf")
            ident = persist.tile([128, 128], BF16, tag="ident")
            biasA = persist.tile([128, 1], F32, tag="biasA")

            # x-tile DMAs are issued first (dma_x below); weights follow on
            # the same queue in need-order, wo (o_proj-only) last.
            def dma_weights_early():
                # base weights on the SP queue, deltas + tables on the DVE
                # queue (parallel transfer engines), in need-order; wq split
                # so the first hc chunks land quickly
                HH = HC2 // 2
                nc.sync.dma_start(out=wk8, in_=wk8d[:, :, :, :])
                nc.sync.dma_start(out=wv8, in_=wv8d[:, :, :, :])
                HQ = HC2 // 4
                nc.sync.dma_start(out=wq8[:, 0:HQ], in_=wq8d[:, 0:HQ, :, :])
                nc.sync.dma_start(out=dwk8, in_=dwk8d[:, :, :, :])
                nc.sync.dma_start(out=wq8[:, HQ:HH], in_=wq8d[:, HQ:HH, :, :])
                nc.sync.dma_start(out=dwv8, in_=dwv8d[:, :, :, :])
                nc.sync.dma_start(out=dwq8[:, 0:HQ],
                                  in_=dwq8d[:, 0:HQ, :, :])
                nc.sync.dma_start(out=dwq8[:, HQ:HH],
                                  in_=dwq8d[:, HQ:HH, :, :])
                nc.sync.dma_start(out=wq8[:, HH:HC2], in_=wq8d[:, HH:HC2, :, :])
                nc.sync.dma_start(out=dwq8[:, HH:HC2],
                                  in_=dwq8d[:, HH:HC2, :, :])
                nc.sync.dma_start(out=cos_sb, in_=cosd[:, :])
                nc.sync.dma_start(out=sin_sb, in_=sind[:, :])

            def dma_weights_late():
                nc.sync.dma_start(out=wo8_sb, in_=wo8d[:, :, :, :])
                nc.sync.dma_start(out=dwo8_sb, in_=dwo8d[:, :, :, :])

            nc.vector.memset(ones8, 1.0)
            nc.vector.memset(ones_bf, 0.03125)
            make_identity(nc, ident)
            nc.vector.memset(biasA, float(PS_SCALE * PS_SCALE * EPS))

            def dma_x(st):
                """Fetch this s-tile's x8/dx8 chunk groups.  Tile 0 rides
                the ACT queue (parallel with the SP weight stream at
                startup); later tiles use SP, whose sequencer is idle
                during attention -- an ACT-queue DMA dispatch costs 667ns
                of the same sequencer that paces exp."""
                ss = bass.ts(st, ST)
                eng = nc.scalar if st == 0 else nc.sync
                x8_g, dx8_g = [], []
                for g in range(HC2 // XG):
                    gx = xin.tile([128, XG, 2, ST], E4, tag="x8", name="x8g")
                    eng.dma_start(
                        out=gx, in_=x8d[:, bass.ts(g, XG), :, ss])
                    x8_g.append(gx)
                    gd = xin.tile([128, XG, 2, ST], E4, tag="dx8", name="dx8g")
                    eng.dma_start(
                        out=gd, in_=dx8d[:, bass.ts(g, XG), :, ss])
                    dx8_g.append(gd)
                return x8_g, dx8_g

            # ---- pass B: projections, two sweeps over the contraction ----
            # sweep1 accumulates {k, v, q0, q1} in 4 banks and carries the
            # norm-stats pass inline (x2 square + ones-DoubleRow per chunk,
            # spread so the PE wait-queue absorbs the square latency); its
            # evacs (and the v transposes) overlap sweep2's {q2, q3}
            # matmuls, so attention head 0 can start while sweep2 is still
            # on the PE and PSUM pressure never exceeds ~6 banks.
            def pass_b(st, x8_g, dx8_g):
                ss = bass.ts(st, ST)

                def sweep(outs, sq_ps=None):
                    # outs: list of (psum, w8, dw8, colslice)
                    n_mm = 3 * HC2
                    i_mm = 0
                    for hc in range(HC2):
                        g, i = hc // XG, hc % XG
                        xs = x8_g[g][:, i, :, :]
                        dxs = dx8_g[g][:, i, :, :]
                        for term in range(3):
                            st_, sp_ = (i_mm == 0), (i_mm == n_mm - 1)
                            for (ps, w8, dw8, cs) in outs:
                                w_st = (w8, dw8, w8)[term]
                                x_mv = (xs, xs, dxs)[term]
                                nc.tensor.matmul(
                                    ps, w_st[:, hc, :, cs], x_mv,
                                    perf_mode=DR, start=st_, stop=sp_)
                            i_mm += 1
                        if sq_ps is not None:
                            x2 = x2p.tile([128, 2, ST], E4, tag="x2",
                                          name="x2")
                            if hc % 2 == 0:
                                nc.vector.scalar_tensor_tensor(
                                    x2, in0=xs, scalar=0.125, in1=xs,
                                    op0=mybir.AluOpType.mult,
                                    op1=mybir.AluOpType.mult)
                            else:
                                nc.scalar.activation(
                                    x2, xs,
                                    mybir.ActivationFunctionType.Square,
                                    scale=0.3535533906)
                            nc.tensor.matmul(sq_ps, ones8, x2, perf_mode=DR,
                                             start=(hc == 0),
                                             stop=(hc == HC2 - 1))

                def rope_evac(src_ps, dst):
                    qb = evacp.tile([128, ST], BF16, tag="qb", name="qb")
                    nc.scalar.copy(qb, src_ps)
                    u = evacp.tile([128, ST], BF16, tag="u", name="u")
                    nc.vector.tensor_mul(
                        u[0:64, :], qb[64:128, :], sp_t[64:128, :])
                    nc.vector.tensor_mul(
                        u[64:128, :], qb[0:64, :], sp_t[0:64, :])
                    nc.vector.tensor_mul(dst, qb, cp_t)
                    nc.vector.tensor_add(dst, dst, u)

                k_ps = acc_ps.tile([128, ST], F32, tag="acc", name="k_ps")
                v_ps = acc_ps.tile([128, ST], F32, tag="acc", name="v_ps")
                q01 = [acc_ps.tile([128, ST], F32, tag="acc", name=f"q_ps{m}")
                       for m in range(2)]
                sq_ps = acc_ps.tile([128, ST], F32, tag="acc", name="sq_ps")
                sweep([(k_ps, wk8, dwk8, slice(0, DKV)),
                       (v_ps, wv8, dwv8, slice(0, DKV)),
                       (q01[0], wq8, dwq8, bass.ts(0, 128)),
                       (q01[1], wq8, dwq8, bass.ts(1, 128))],
                      sq_ps=sq_ps)
                if stop_after == "sweep1":
                    nc.scalar.copy(kT[:, ss], k_ps)
                    nc.scalar.copy(vT[:, ss], v_ps)
                    nc.scalar.copy(qT[:, 0, ss], q01[0])
                    nc.scalar.copy(qT[:, 1, ss], q01[1])
                    sd0 = tabs.tile([128, ST], F32, tag="sd", name="sd")
                    nc.scalar.activation(
                        sd0, sq_ps, mybir.ActivationFunctionType.Sqrt,
                        scale=float(PS_SCALE * PS_SCALE / (8.0 * h)),
                        bias=biasA)
                    return
                # norm-stat tables: R = 1/(16384*sqrt(mean(x^2)+eps)) =
                # exp(-0.5*ln(16384^2*(mean+eps))) -- ln and exp live in the
                # same ACT table set as Copy/Square, so no table reloads
                # (sqrt would force two per tile); broadcast on all
                # partitions already
                sd = tabs.tile([128, ST], F32, tag="sd", name="sd")
                nc.scalar.activation(
                    sd, sq_ps, mybir.ActivationFunctionType.Ln,
                    scale=float(PS_SCALE * PS_SCALE / (8.0 * h)),
                    bias=biasA)
                R_t = tabs.tile([128, ST], F32, tag="R", name="R_t")
                nc.scalar.activation(
                    R_t, sd, mybir.ActivationFunctionType.Exp, scale=-0.5)
                if debug:
                    nc.sync.dma_start(out=dbg["dbg_R"][:, ss], in_=R_t)
                cp_t = tabs.tile([128, ST], BF16, tag="cp", name="cp_t")
                nc.vector.tensor_mul(cp_t, cos_sb[:, ss], R_t)
                sp_t = tabs.tile([128, ST], BF16, tag="sp", name="sp_t")
                nc.vector.tensor_mul(sp_t, sin_sb[:, ss], R_t)
                rope_evac(k_ps, kT[:, ss])
                rope_evac(q01[0], qT[:, 0, ss])
                vb = evacp.tile([128, ST], BF16, tag="qb", name="vb")
                nc.scalar.copy(vb, v_ps)
                nc.vector.tensor_mul(vT[:, ss], vb, R_t)
                q23 = [acc_ps.tile([128, ST], F32, tag="acc", name=f"q_ps{m}")
                       for m in (2, 3)]
                sweep([(q23[0], wq8, dwq8, bass.ts(2, 128)),
                       (q23[1], wq8, dwq8, bass.ts(3, 128))])
                # transpose v into [j, d] layout (overlaps sweep2)
                for jc in range(st * (ST // 128), (st + 1) * (ST // 128)):
                    vt_ps = acc_ps.tile([128, 128], BF16, tag="acc",
                                        name="vt_ps")
                    nc.tensor.transpose(vt_ps, vT[:, bass.ts(jc, 128)], ident)
                    nc.scalar.copy(vnat[:, jc, :], vt_ps)
                rope_evac(q01[1], qT[:, 1, ss])
                rope_evac(q23[0], qT[:, 2, ss])
                rope_evac(q23[1], qT[:, 3, ss])

            # ---- attention per (head, i-tile), diagonal-trimmed ----
            # filler: generator of o_proj ht-tile emitters, consumed between
            # attention chunks to keep the PE fed during ACT exp waits
            def attn_head(m, ti, filler=None):
                iss = bass.ts(ti, ST)
                njc = (ti + 1) * (ST // 128)
                av_ps = acc_ps.tile([128, ST], F32, tag="acc", name="av_ps")
                z_ps = acc_ps.tile([128, ST], F32, tag="acc", name="z_ps")
                for jc in range(njc):
                    if filler is not None and jc % 2 == 1:
                        next(filler, None)
                    off = 0 if no_trim else max(0, jc * 128 - ti * ST)
                    s_ps = acc_ps.tile([128, ST], F32, tag="acc", name="s_ps")
                    nc.tensor.matmul(
                        s_ps[:, off:ST], kT[:, bass.ts(jc, 128)],
                        qT[:, m, ti * ST + off:(ti + 1) * ST],
                        start=True, stop=True)
                    p = probs.tile([128, ST], BF16, tag="p", name="p")
                    nc.scalar.activation(
                        p[:, off:ST], s_ps[:, off:ST],
                        mybir.ActivationFunctionType.Exp, scale=scale)
                    diag = (jc + 1) * 128 > ti * ST
                    if diag and no_trim:
                        nc.gpsimd.affine_select(
                            out=p, in_=p, pattern=[[1, ST]],
                            compare_op=mybir.AluOpType.is_ge,
                            fill=0.0, base=ti * ST - jc * 128,
                            channel_multiplier=-1)
                    elif diag:
                        nc.gpsimd.affine_select(
                            out=p[:, off:off + 128], in_=p[:, off:off + 128],
                            pattern=[[1, 128]],
                            compare_op=mybir.AluOpType.is_ge,
                            fill=0.0, base=0, channel_multiplier=-1)
                    nc.tensor.matmul(av_ps[:, off:ST], vnat[:, jc, :],
                                     p[:, off:ST],
                                     start=(jc == 0), stop=(jc == njc - 1))
                    nc.tensor.matmul(z_ps[:, off:ST], ones_bf, p[:, off:ST],
                                     start=(jc == 0), stop=(jc == njc - 1))
                zr = zrp.tile([128, ST], F32, tag="zr", name="zr")
                nc.vector.reciprocal(zr, z_ps)
                t_bf = zrp.tile([128, ST], BF16, tag="tbf", name="t_bf")
                nc.vector.tensor_mul(t_bf, av_ps, zr)      # 32*attn, bf16
                a8s = attnT8[:, m // 2, m % 2, iss]
                nc.vector.tensor_copy(a8s, t_bf)
                nc.vector.scalar_tensor_tensor(
                    dattnT8[:, m // 2, m % 2, iss], in0=a8s, scalar=-1.0,
                    in1=t_bf, op0=mybir.AluOpType.mult,
                    op1=mybir.AluOpType.add)

            # ---- o_proj for one 128-row s-chunk (generator: one ht-tile
            # per next(), so attention can interleave it as PE filler) ----
            def o_proj_gen(sc):
                scs = bass.ts(sc, 128)
                for half in range(2):
                    o_sb = outp.tile([128, h // 2], BF16, tag="osb",
                                     name="o_sb")
                    for hi in range(NHT // 2):
                        ht = half * (NHT // 2) + hi
                        hts = bass.ts(ht, 512)
                        o_ps = acc_ps.tile([128, 512], F32, tag="acc",
                                           name="o_ps")
                        i_mm = 0
                        for c in range(2):
                            a_s = attnT8[:, c, :, scs]
                            d_s = dattnT8[:, c, :, scs]
                            w_s = wo8_sb[:, c, :, hts]
                            dw_s = dwo8_sb[:, c, :, hts]
                            for stat, mov in ((a_s, w_s), (a_s, dw_s),
                                              (d_s, w_s)):
                                nc.tensor.matmul(
                                    o_ps, stat, mov, perf_mode=DR,
                                    start=(i_mm == 0),
                                    stop=(i_mm == 5 and c == 1))
                                i_mm += 1
                        # gpsimd cannot read PSUM; alternate DVE/ACT --
                        # with the fp8 attn evac chain DVE is the loaded
                        # engine in attention windows
                        dst = o_sb[:, bass.ts(hi, 512)]
                        if ht % 2 == 1:
                            nc.vector.tensor_scalar_mul(dst, o_ps,
                                                        1.0 / 65536.0)
                        else:
                            nc.scalar.activation(
                                dst, o_ps,
                                mybir.ActivationFunctionType.Copy,
                                scale=1.0 / 65536.0)
                        yield
                    nc.sync.dma_start(
                        out=outd[scs, bass.ts(half, h // 2)], in_=o_sb)
                while True:
                    yield

            def o_proj_chunk(sc):
                g = o_proj_gen(sc)
                for _ in range(NHT + 1):
                    next(g)

            # ---- orchestrate ----
            # o_proj chunks of tile st-1 are interleaved between the heads of
            # tile st's attention: their matmuls have no unresolved deps, so
            # they fill PE stalls while attention waits on ACT exp.
            xg = dma_x(0)
            dma_weights_early()
            dma_weights_late()
            for st in range(NST):
                pass_b(st, xg[0], xg[1])
                if st + 1 < NST:
                    xg = dma_x(st + 1)
                if stop_after == "proj":
                    continue
                for m in range(qh):
                    if st > 0 and stop_after != "attn":
                        o_proj_chunk((st - 1) * (ST // 128) + m)
                    attn_head(m, st)
            if stop_after is None:
                for sc in range((NST - 1) * (ST // 128), NST * (ST // 128)):
                    o_proj_chunk(sc)
            if debug:
                nc.sync.dma_start(out=dbg["dbg_qT"][:, :, :], in_=qT)
                nc.sync.dma_start(out=dbg["dbg_kT"][:, :], in_=kT)
                nc.sync.dma_start(out=dbg["dbg_vT"][:, :], in_=vT)
                nc.sync.dma_start(out=dbg["dbg_vnat"][:, :, :], in_=vnat)

    nc.compile()
    return nc


def make_core_inputs(hidden_states, cos, sin, norm_w, wq, wk, wv, wo,
                     s=S, h=H, qh=QH, n_cores=N_CORES):
    """Host-side sharding + fp8 delta decomposition + layout prep."""
    import ml_dtypes

    E4NP = ml_dtypes.float8_e4m3
    BFNP = ml_dtypes.bfloat16
    HC2 = h // 256
    dq = qh * HD

    def to_chunks(a, cols):
        # [h, cols] -> [128, HC2, 2, cols] with h = hc*256 + t*128 + p
        return np.ascontiguousarray(
            a.reshape(HC2, 2, 128, cols).transpose(2, 0, 1, 3))

    x = np.asarray(hidden_states, np.float32).reshape(s, h)
    nw = np.asarray(norm_w, np.float32)
    xT = np.ascontiguousarray(x.T)                       # [h, s]
    x8 = (xT * SX).astype(E4NP)
    dx8 = ((xT * SX - x8.astype(np.float32)) * FX).astype(E4NP)
    x8 = to_chunks(x8, s)
    dx8 = to_chunks(dx8, s)

    cosT = np.asarray(cos, np.float32).reshape(s, HD).T   # [HD, s]
    sinT = np.asarray(sin, np.float32).reshape(s, HD).T
    sinTs = np.concatenate([sinT[64:128], -sinT[0:64]], axis=0)
    cosT = np.ascontiguousarray(cosT.astype(BFNP))
    sinTs = np.ascontiguousarray(sinTs.astype(BFNP))

    wq_f = np.asarray(wq, np.float32) * nw[:, None]
    wk_f = np.asarray(wk, np.float32) * nw[:, None]
    wv_f = np.asarray(wv, np.float32) * nw[:, None]
    wo_f = np.asarray(wo, np.float32)

    def w_split(w):
        w8 = (w * SW).astype(E4NP)
        dw8 = ((w * SW - w8.astype(np.float32)) * FW).astype(E4NP)
        return w8, dw8

    in_maps = []
    for c in range(n_cores):
        wq_c = wq_f[:, c * dq:(c + 1) * dq]
        wk_c = wk_f[:, c * DKV:(c + 1) * DKV]
        wv_c = wv_f[:, c * DKV:(c + 1) * DKV]
        wq8, dwq8 = w_split(wq_c)
        wk8, dwk8 = w_split(wk_c)
        wv8, dwv8 = w_split(wv_c)
        wo_c = wo_f[c * dq:(c + 1) * dq, :] * SW         # [512, h]
        wo8 = wo_c.astype(E4NP)
        dwo8 = (wo_c - wo8.astype(np.float32)).astype(E4NP)
        wo8 = np.ascontiguousarray(
            wo8.reshape(2, 2, 128, h).transpose(2, 0, 1, 3))
        dwo8 = np.ascontiguousarray(
            dwo8.reshape(2, 2, 128, h).transpose(2, 0, 1, 3))
        in_maps.append({
            "x8": x8, "dx8": dx8,
            "wq8": to_chunks(wq8, dq), "dwq8": to_chunks(dwq8, dq),
            "wk8": to_chunks(wk8, DKV), "dwk8": to_chunks(dwk8, DKV),
            "wv8": to_chunks(wv8, DKV), "dwv8": to_chunks(dwv8, DKV),
            "wo8": wo8, "dwo8": dwo8, "cosT": cosT, "sinTs": sinTs,
        })
    return in_maps


_NC_CACHE = {}


def kernel(hidden_states, cos, sin, norm_w, wq, wk, wv, wo):
    from concourse.bass_utils import run_bass_kernel_spmd

    if "nc" not in _NC_CACHE:
        _NC_CACHE["nc"] = build_bass()
    nc = _NC_CACHE["nc"]
    in_maps = make_core_inputs(hidden_states, cos, sin, norm_w, wq, wk, wv, wo)
    res = run_bass_kernel_spmd(nc, in_maps, core_ids=list(range(N_CORES)))
    out = np.asarray(hidden_states, np.float32).reshape(S, H).copy()
    for m in res.results:
        out += np.asarray(m["out"], np.float32)
    return out.reshape(B, S, H)
